# revision 39
# baseline (speedup 1.0000x reference)
"""Trainium2 Bass kernel for MinimalEventMamba (v2).

kernel(**inputs) takes FULL inputs (as from setup_inputs()) and returns the
FULL (4, 10, 64, 64) float32 output. Batch-parallel across 8 NeuronCores
(4 batches x2 replicated, state dim split 8/8 across each pair), one SPMD
Bass launch.

v2 structure (vs v1 baseline at ~1.02ms):
- per-layer work split in sequence halves; front-end (in_proj/dwconv/silu,
  x_proj, softplus via sigmoid+ln) for half h is emitted right before half
  h's scans, so it executes under the previous half's scans.
- scan block is hf-outer / s-inner with per-state carry columns. Per-state
  y contributions (hs*C_s) are accumulated on the TensorEngine via
  identity-matmul PSUM accumulation seeded with the D*u term (no DVE adds);
  gating by silu(z) + out_proj + pair-AllReduce run per quarter and are
  deferred into the next half's scan window (CC flush at s==0, residual
  flush at s==4) so the collective latency hides under scans.
- xs = du*B_s runs on GpSimd; DVE does only scans, hc, yg, du, residual.
- act-table thrash avoided by staging per half: silu -> sigmoid -> ln -> exp.
- encoder convolves only the core's own batch; BN stats via one 8-rank
  AllReduce. decoder convs use K=128 tap-pairing (5 matmuls per 9 taps).
"""
import sys
import types

sys.path.insert(0, "/opt/trn_rl_repo")
sys.path.insert(0, "/opt/trn_rl_repo/concourse")
try:
    from antenv import axon_hooks  # noqa: F401
except ImportError:
    try:
        from trn_agent_boot.trn_boot import _ntff_profile_via_ctypes
        _m = types.ModuleType("antenv.axon_hooks")
        _h = _ntff_profile_via_ctypes("/opt/axon/libaxon_pjrt.so")
        _m.get_axon_ntff_profile_hook = lambda: _h
        _m.set_axon_ntff_profile_hook = lambda h: None
        sys.modules["antenv.axon_hooks"] = _m
    except Exception:
        pass

from contextlib import ExitStack

import numpy as np
import ml_dtypes

import concourse.bass as bass
import concourse.tile as tile
from concourse import mybir
from concourse.bass_utils import run_bass_kernel_spmd
import bass_rust

F32 = mybir.dt.float32
BF16 = mybir.dt.bfloat16
FP16 = mybir.dt.float16

NB, HD, NL, NF = 5, 64, 4, 10
DI, DS, DC, DTR = 128, 16, 4, 4
B, H, W = 4, 64, 64
L = H * W                     # 4096
PW = W + 2                    # padded row stride 66
PADL = PW * (H + 2) + 4       # padded spatial + guard cols (4360)
PBASE = 1 + PW + 1            # first interior col in padded layout
CS = 512
LH = L // 2                   # scan half length
QS = L // 4                   # gate/project/AllReduce quarter
DSL = DS // 2                 # states per core (s-split across core pairs)
XS_ON_GPSIMD = False          # Pool engine can't codegen TensorTensor here


def split_excess_waits(nc, max_waits=1):
    """This container's walrus accepts only 1 sync wait per instruction;
    move overflow waits onto NOPs inserted before the offending op."""
    f = nc.m.functions[0]
    for bb in f.blocks:
        insts = bb.instructions
        i = 0
        while i < len(insts):
            inst = insts[i]
            si = inst.sync_info
            if si is not None and len(si.on_wait) > max_waits:
                waits = list(si.on_wait)
                si.on_wait = waits[-max_waits:]
                inst.sync_info = si
                overflow = waits[:-max_waits]
                eng = nc.engines[inst.engine]
                pos = i
                for j in range(0, len(overflow), max_waits):
                    nop = eng.nop(hint="splitw", nofuse=True)
                    nop_inst = nop.ins if hasattr(nop, "ins") else nop
                    for bb2 in f.blocks:
                        if any(x is nop_inst for x in bb2.instructions):
                            bb2.instructions[:] = [
                                x for x in bb2.instructions if x is not nop_inst
                            ]
                            break
                    nop_inst.sync_info = bass_rust.SyncInfo(
                        on_wait=overflow[j : j + max_waits], on_update=[]
                    )
                    insts.insert(pos, nop_inst)
                    pos += 1
                i = pos + 1
            else:
                i += 1


def build_kernel():
    nc = bass.Bass()
    dp = nc.declare_dram_parameter

    enc_in = dp("enc_im2col", [45, B * L], FP16, isOutput=False)
    enc_w2 = dp("enc_w2", [45, HD], FP16, isOutput=False)
    enc_g = dp("enc_g", [HD, 1], F32, isOutput=False)
    enc_be = dp("enc_be", [HD, 1], F32, isOutput=False)
    ip_tap = dp("ip_tap", [HD, NL * DC * DI], FP16, isOutput=False)
    ip_z = dp("ip_z", [HD, NL * DI], FP16, isOutput=False)
    conv_b = dp("conv_b", [DI, NL], F32, isOutput=False)
    wd_T = dp("wd_T", [DI, NL * DI], BF16, isOutput=False)
    bc_T = dp("bc_T", [DI, NL * 2 * DSL], BF16, isOutput=False)
    dt_b = dp("dt_b", [DI, NL], F32, isOutput=False)
    a_cols = dp("a_cols", [DI, NL * DSL], F32, isOutput=False)
    d_col = dp("d_col", [DI, NL], F32, isOutput=False)
    op_T = dp("op_T", [DI, NL * HD], BF16, isOutput=False)
    ident_p = dp("ident", [DI, DI], BF16, isOutput=False)
    dec1_tap = dp("dec1_tap", [2 * HD, 5 * HD], FP16, isOutput=False)
    dec1_g = dp("dec1_g", [HD, 1], F32, isOutput=False)
    dec1_be = dp("dec1_be", [HD, 1], F32, isOutput=False)
    dec2_tap = dp("dec2_tap", [2 * HD, 5 * NF], FP16, isOutput=False)
    dec2_b = dp("dec2_b", [NF, 1], F32, isOutput=False)

    out_ext = dp("out", [NF, L], F32, isOutput=True)

    bc_dram = nc.dram_tensor("bc_dram", [NL, 2 * DSL, L], BF16)
    y_in = nc.dram_tensor("y_in", [2, HD, LH], BF16)
    y_out = nc.dram_tensor("y_out", [2, HD, LH], BF16)
    cc_in = nc.dram_tensor("cc_in", [HD, 2], F32)
    cc_out = nc.dram_tensor("cc_out", [HD, 2], F32, addr_space="Shared")
    cc2_in = nc.dram_tensor("cc2_in", [HD, 2], F32)
    cc2_out = nc.dram_tensor("cc2_out", [HD, 2], F32, addr_space="Shared")

    ctx = ExitStack()
    with ctx:
        tc = ctx.enter_context(tile.TileContext(nc))
        const = ctx.enter_context(tc.tile_pool(name="const", bufs=1))
        persist = ctx.enter_context(tc.tile_pool(name="persist", bufs=1))
        work = ctx.enter_context(tc.tile_pool(name="work", bufs=1))
        stream = ctx.enter_context(tc.tile_pool(name="stream", bufs=2))
        sloop = ctx.enter_context(tc.tile_pool(name="sloop", bufs=2))
        small = ctx.enter_context(tc.tile_pool(name="small", bufs=1))
        # PSUM: py 4 banks + fe 2x1 + proj 2 = 8 banks exactly
        psum = ctx.enter_context(tc.tile_pool(name="psum", bufs=2, space="PSUM"))
        psumy = ctx.enter_context(tc.tile_pool(name="psumy", bufs=1, space="PSUM"))

        MM = nc.tensor.matmul
        AF = mybir.ActivationFunctionType
        OP = mybir.AluOpType
        X = mybir.AxisListType

        # ---------------- encoder (own batch only) ----------------
        enc_w_t = const.tile([45, HD], FP16)
        nc.sync.dma_start(enc_w_t[:], enc_w2[:])
        enc_g_t = const.tile([HD, 1], F32)
        nc.sync.dma_start(enc_g_t[:], enc_g[:])
        enc_be_t = const.tile([HD, 1], F32)
        nc.sync.dma_start(enc_be_t[:], enc_be[:])

        # all-batch conv on every core: exact BN stats with no collective
        # (own batch's chunks come first in the host-side im2col layout)
        enc_keep = work.tile([HD, L], F32, tag="big")
        s1p = small.tile([HD, 32], F32, tag="s1p")
        s2p = small.tile([HD, 32], F32, tag="s2p")
        for n in range(32):
            cin = stream.tile([45, CS], FP16, tag="enc_cin", bufs=6)
            nc.sync.dma_start(cin[:], enc_in[:, bass.ts(n, CS)])
            pt = psum.tile([HD, CS], F32,
                           tag=("fe" if n % 3 else "proj"),
                           bufs=(2 if n % 3 else 1))
            MM(pt[:], enc_w_t[:], cin[:], start=True, stop=True)
            if n < 8:
                dst = enc_keep[:, bass.ts(n, CS)]
            else:
                scr = stream.tile([HD, CS], F32, tag="enc_scr")
                dst = scr[:]
            nc.scalar.activation(dst, pt[:], AF.Copy,
                                 accum_out=s1p[:, n : n + 1])
            sq = stream.tile([HD, CS], F32, tag="enc_sq")
            nc.vector.scalar_tensor_tensor(sq[:], dst, 1.0, dst,
                                           OP.mult, OP.mult,
                                           accum_out=s2p[:, n : n + 1])
        red0 = small.tile([HD, 2], F32, tag="red0")
        nc.vector.tensor_reduce(red0[:, 0:1], s1p[:], axis=X.X, op=OP.add)
        nc.vector.tensor_reduce(red0[:, 1:2], s2p[:], axis=X.X, op=OP.add)

        def bn_scale_bias(s1ap, s2ap, n_elems, g_ap, be_ap, tag):
            inv_n = 1.0 / n_elems
            mean = small.tile([HD, 1], F32, tag=tag + "m")
            nc.vector.tensor_scalar_mul(mean[:], s1ap, inv_n)
            m2 = small.tile([HD, 1], F32, tag=tag + "m2")
            nc.vector.tensor_tensor(m2[:], mean[:], mean[:], OP.mult)
            var = small.tile([HD, 1], F32, tag=tag + "v")
            nc.vector.scalar_tensor_tensor(var[:], s2ap, inv_n, m2[:],
                                           OP.mult, OP.subtract)
            veps = small.tile([HD, 1], F32, tag=tag + "ve")
            nc.vector.tensor_scalar_add(veps[:], var[:], 1e-5)
            rv = small.tile([HD, 1], F32, tag=tag + "rv")
            nc.vector.reciprocal(rv[:], veps[:])
            rstd = small.tile([HD, 1], F32, tag=tag + "rs")
            nc.scalar.activation(rstd[:], rv[:], AF.Sqrt)
            scale = small.tile([HD, 1], F32, tag=tag + "sc")
            nc.vector.tensor_tensor(scale[:], g_ap, rstd[:], OP.mult)
            nscale = small.tile([HD, 1], F32, tag=tag + "ns")
            nc.vector.tensor_scalar_mul(nscale[:], scale[:], -1.0)
            bias = small.tile([HD, 1], F32, tag=tag + "bi")
            nc.vector.scalar_tensor_tensor(bias[:], mean[:], nscale[:], be_ap,
                                           OP.mult, OP.add)
            return scale, bias

        sc0, bi0 = bn_scale_bias(red0[:, 0:1], red0[:, 1:2], B * L,
                                 enc_g_t[:], enc_be_t[:], "bn0")

        t_t = persist.tile([HD, 4 + L], F32)
        nc.vector.memset(t_t[:, 0:4], 0.0)
        t16 = persist.tile([HD, 4 + L], FP16)
        nc.vector.memset(t16[:, 0:4], 0.0)
        for hh in range(2):
            hsl = slice(4 + hh * LH, 4 + (hh + 1) * LH)
            nc.scalar.activation(t_t[:, hsl], enc_keep[:, hh * LH : (hh + 1) * LH],
                                 AF.Relu, bias=bi0[:], scale=sc0[:])
            nc.scalar.activation(t16[:, hsl], t_t[:, hsl], AF.Copy)

        # ---------------- trunk weights ----------------
        iptap_t = const.tile([HD, NL * DC * DI], FP16)
        nc.sync.dma_start(iptap_t[:], ip_tap[:])
        ipz_t = const.tile([HD, NL * DI], FP16)
        nc.sync.dma_start(ipz_t[:], ip_z[:])
        convb_t = const.tile([DI, NL], F32)
        nc.sync.dma_start(convb_t[:], conv_b[:])
        wd_t = const.tile([DI, NL * DI], BF16)
        nc.sync.dma_start(wd_t[:], wd_T[:])
        bct_t = const.tile([DI, NL * 2 * DSL], BF16)
        nc.sync.dma_start(bct_t[:], bc_T[:])
        dtb_t = const.tile([DI, NL], F32)
        nc.sync.dma_start(dtb_t[:], dt_b[:])
        acols_t = const.tile([DI, NL * DSL], F32)
        nc.sync.dma_start(acols_t[:], a_cols[:])
        dcol_t = const.tile([DI, NL], F32)
        nc.sync.dma_start(dcol_t[:], d_col[:])
        opt_t = const.tile([DI, NL * HD], BF16)
        nc.sync.dma_start(opt_t[:], op_T[:])
        ident_t = const.tile([DI, DI], BF16)
        nc.sync.dma_start(ident_t[:], ident_p[:])

        # ---------------- trunk ----------------
        # deferred per-quarter tails: projection+evict+CC flush at the next
        # half's s==0 (PE/Act/Pool streams run them under the scans);
        # dtsum+residual+t16 at the next half's s==5 (AR has landed by then)
        pend_cc = []
        pend_dma = []
        pend_post = []

        def flush(lst):
            for fn in lst:
                fn()
            lst.clear()

        for li in range(NL):
            xi_c = work.tile([DI, L], BF16, tag="xi_c")
            sz = work.tile([DI, L], BF16, tag="sz")
            dlt = work.tile([DI, L], BF16, tag="dlt")
            du = work.tile([DI, L], BF16, tag="du")
            bc_sb = work.tile([2 * DSL, L], BF16, tag="bc_sb")
            dt_part = work.tile([HD, L], BF16, tag="dt_part")
            dtsum = work.tile([HD, L], BF16, tag="dtsum")
            carry = small.tile([DI, DSL], BF16, tag="carry")

            for hf in range(2):
                sl = slice(hf * LH, (hf + 1) * LH)
                # ---- front-end for this half (chunks 4hf .. 4hf+3) ----
                # stage A: silu table
                for n in range(4 * hf, 4 * hf + 4):
                    p_xi = psum.tile([DI, CS], F32, tag="fe")
                    for k in range(DC):
                        MM(p_xi[:],
                           iptap_t[:, (li * DC + k) * DI : (li * DC + k + 1) * DI],
                           t16[:, 1 + k + n * CS : 1 + k + n * CS + CS],
                           start=(k == 0), stop=(k == DC - 1))
                    nc.scalar.activation(xi_c[:, bass.ts(n, CS)], p_xi[:],
                                         AF.Silu,
                                         bias=convb_t[:, li : li + 1], scale=1.0)
                # z path off the critical chain: xi feeds FE-B immediately,
                # sz is only needed at gating time (end of the scan block)
                for n in range(4 * hf, 4 * hf + 4):
                    p_z = psum.tile([DI, CS], F32, tag="fe")
                    MM(p_z[:], ipz_t[:, li * DI : (li + 1) * DI],
                       t16[:, 4 + n * CS : 4 + (n + 1) * CS],
                       start=True, stop=True)
                    nc.scalar.activation(sz[:, bass.ts(n, CS)], p_z[:], AF.Silu)
                # stage B: sigmoid table (+ bc Copy evictions)
                sg = work.tile([DI, LH], F32, tag="big")
                for j in range(4):
                    n = 4 * hf + j
                    p_d = psum.tile([DI, CS], F32, tag="fe")
                    MM(p_d[:], wd_t[:, li * DI : (li + 1) * DI],
                       xi_c[:, bass.ts(n, CS)], start=True, stop=True)
                    nc.scalar.activation(sg[:, bass.ts(j, CS)], p_d[:],
                                         AF.Sigmoid,
                                         bias=dtb_t[:, li : li + 1], scale=-1.0)
                    p_bc = psum.tile([2 * DSL, CS], F32, tag="fe")
                    MM(p_bc[:], bct_t[:, li * 2 * DSL : (li + 1) * 2 * DSL],
                       xi_c[:, bass.ts(n, CS)], start=True, stop=True)
                    nc.scalar.activation(bc_sb[:, bass.ts(n, CS)], p_bc[:],
                                         AF.Copy)
                    nc.sync.dma_start(bc_dram[li][:, bass.ts(n, CS)],
                                      bc_sb[:, bass.ts(n, CS)])
                # D*u seed early (Copy shares the sigmoid table, and keeps
                # the dterm copy off the ln->exp critical tail)
                dterm = sloop.tile([DI, LH], BF16, tag="dterm", bufs=1)
                nc.scalar.activation(dterm[:], xi_c[:, sl], AF.Copy,
                                     scale=dcol_t[:, li : li + 1])
                # stage C: ln table; du on DVE
                for j in range(4):
                    n = 4 * hf + j
                    nc.scalar.activation(dlt[:, bass.ts(n, CS)],
                                         sg[:, bass.ts(j, CS)], AF.Ln)
                    nc.vector.tensor_tensor(du[:, bass.ts(n, CS)],
                                            dlt[:, bass.ts(n, CS)],
                                            xi_c[:, bass.ts(n, CS)], OP.mult)

                # ---- scan block ----
                # previous half's projection + exchange first: its Act/PE ops
                # land ahead of dterm/exp so the AllReduce input chain starts
                # as early as possible (AR end-to-end latency is ~40us)
                flush(pend_cc)
                # id-matmul accumulation covers dterm + states 0..5; the last
                # two states fold into yg on DVE so late idMMs don't block
                # the next half's FE matmuls on the in-order PE queue.
                p_y = psumy.tile([DI, LH], F32, tag="py")
                for c in range(LH // CS):
                    MM(p_y[:, bass.ts(c, CS)], ident_t[:],
                       dterm[:, bass.ts(c, CS)], start=True, stop=False)
                hc_last = [None, None]
                for s in range(DSL):
                    dA = sloop.tile([DI, LH], FP16, tag="dA", bufs=4)
                    nc.scalar.activation(
                        dA[:], dlt[:, sl], AF.Exp,
                        scale=acols_t[:, li * DSL + s : li * DSL + s + 1])
                    brep = sloop.tile([DI, LH], BF16, tag="brep", bufs=3)
                    for ph in range(2):
                        nc.sync.dma_start(
                            brep[64 * ph : 64 * ph + 64, :],
                            bc_dram[li][s : s + 1, sl].broadcast_to((64, LH)))
                    crep = sloop.tile([DI, LH], BF16, tag="crep", bufs=2)
                    for ph in range(2):
                        nc.sync.dma_start(
                            crep[64 * ph : 64 * ph + 64, :],
                            bc_dram[li][DSL + s : DSL + s + 1, sl].broadcast_to(
                                (64, LH)))
                    xs = sloop.tile([DI, LH], BF16, tag="xs")
                    nc.vector.tensor_tensor(xs[:], du[:, sl], brep[:], OP.mult)
                    hs = sloop.tile([DI, LH], BF16, tag="hs")
                    init = 0.0 if hf == 0 else carry[:, s : s + 1]
                    nc.vector.tensor_tensor_scan(hs[:], dA[:], xs[:], init,
                                                 OP.mult, OP.add)
                    if hf == 0:
                        nc.vector.tensor_copy(carry[:, s : s + 1],
                                              hs[:, LH - 1 : LH])
                    if s < DSL - 2:
                        hc = sloop.tile([DI, LH], BF16, tag="hc")
                        nc.vector.tensor_tensor(hc[:], hs[:], crep[:], OP.mult)
                        for c in range(LH // CS):
                            MM(p_y[:, bass.ts(c, CS)], ident_t[:],
                               hc[:, bass.ts(c, CS)], start=False,
                               stop=(s == DSL - 3))
                    else:
                        hc = sloop.tile([DI, LH], BF16, tag="hcl", bufs=2)
                        nc.vector.tensor_tensor(hc[:], hs[:], crep[:], OP.mult)
                        hc_last[s - (DSL - 2)] = hc
                    if s == 5:
                        flush(pend_dma)
                    if s == 6:
                        flush(pend_post)
                # ---- gate (+ last two states) per quarter, then one
                # projection + AllReduce per half (fewer collectives: the CC
                # pipeline is latency-bound, ~20us per collective) ----
                ygs = []
                for qq in range(2):
                    q = 2 * hf + qq
                    qs = slice(q * QS, (q + 1) * QS)
                    cq = slice(qq * QS, (qq + 1) * QS)
                    ysum = sloop.tile([DI, QS], BF16, tag="ysum")
                    nc.vector.scalar_tensor_tensor(ysum[:], p_y[:, cq], 1.0,
                                                   hc_last[0][:, cq],
                                                   OP.mult, OP.add)
                    nc.vector.tensor_tensor(ysum[:], ysum[:],
                                            hc_last[1][:, cq], OP.add)
                    yg = sloop.tile([DI, QS], BF16, tag="yg")
                    nc.vector.tensor_tensor(yg[:], ysum[:], sz[:, qs], OP.mult)
                    ygs.append(yg)

                def mk_cc(li=li, hf=hf, sl=sl, ygs=ygs, dtsum=dtsum,
                          dt_part=dt_part):
                    def go():
                        for qq in range(2):
                            q = 2 * hf + qq
                            qs = slice(q * QS, (q + 1) * QS)
                            p_o = psum.tile([HD, QS], F32, tag="proj", bufs=1,
                                            name=f"p_o_{li}_{q}")
                            for c in range(QS // CS):
                                MM(p_o[:, bass.ts(c, CS)],
                                   opt_t[:, li * HD : (li + 1) * HD],
                                   ygs[qq][:, bass.ts(c, CS)],
                                   start=True, stop=True)
                            nc.scalar.activation(dt_part[:, qs], p_o[:],
                                                 AF.Copy)
                        for rb in range(4):
                            nc.sync.dma_start(
                                y_in[hf][16 * rb : 16 * rb + 16, :],
                                dt_part[16 * rb : 16 * rb + 16, sl])
                        nc.gpsimd.collective_compute(
                            "AllReduce", OP.add,
                            replica_groups=[[0, 4], [1, 5], [2, 6], [3, 7]],
                            ins=[y_in[hf]], outs=[y_out[hf]])
                    return go

                def mk_dma(hf=hf, sl=sl, dtsum=dtsum):
                    def go():
                        # the AR is nearly done by now: minimal ring parking
                        for rb in range(4):
                            nc.sync.dma_start(
                                dtsum[16 * rb : 16 * rb + 16, sl],
                                y_out[hf][16 * rb : 16 * rb + 16, :])
                    return go

                def mk_post(li=li, hf=hf, sl=sl, dtsum=dtsum):
                    def go():
                        tsl = slice(4 + hf * LH, 4 + (hf + 1) * LH)
                        nc.vector.tensor_tensor(t_t[:, tsl], t_t[:, tsl],
                                                dtsum[:, sl], OP.add)
                        if li < NL - 1:
                            nc.scalar.activation(t16[:, tsl], t_t[:, tsl],
                                                 AF.Copy)
                    return go

                pend_cc.append(mk_cc())
                pend_dma.append(mk_dma())
                pend_post.append(mk_post())

        flush(pend_cc)
        flush(pend_dma)
        flush(pend_post)

        # ---------------- decoder ----------------
        d1_taps = const.tile([2 * HD, 5 * HD], FP16)
        nc.sync.dma_start(d1_taps[:], dec1_tap[:])
        d2_taps = const.tile([2 * HD, 5 * NF], FP16)
        nc.sync.dma_start(d2_taps[:], dec2_tap[:])
        d1g_t = const.tile([HD, 1], F32)
        nc.sync.dma_start(d1g_t[:], dec1_g[:])
        d1be_t = const.tile([HD, 1], F32)
        nc.sync.dma_start(d1be_t[:], dec1_be[:])
        d2b_t = const.tile([NF, 1], F32)
        nc.sync.dma_start(d2b_t[:], dec2_b[:])

        # padA/padB: rows 0:HD = base, rows HD:2HD = base shifted +2 cols.
        # padA3/padB3: rows 0:HD = base copy, rows HD:2HD = base shifted +132.
        padA = work.tile([2 * HD, PADL], FP16, tag="xi_c")
        nc.vector.memset(padA[0:HD, :], 0.0)
        padA3 = work.tile([2 * HD, PADL], FP16, tag="sz")
        padB = work.tile([HD, PADL], FP16, tag="dt_part")
        out_pad = work.tile([NF, PADL], F32, tag="big")

        def interior(tile_ap):
            return tile_ap[0:HD, PBASE : PBASE + PW * H].rearrange(
                "p (h w) -> p h w", w=PW)[:, :, 0:W]

        nc.scalar.activation(interior(padA),
                             t_t[:, 4:].rearrange("p (h w) -> p h w", w=W),
                             AF.Copy)

        def mk_pair_copies(base, base3):
            nc.vector.tensor_copy(base[HD : 2 * HD, 0 : PADL - 2],
                                  base[0:HD, 2:PADL])
            nc.vector.memset(base[HD : 2 * HD, PADL - 2 : PADL], 0.0)
            nc.vector.tensor_copy(base3[0:HD, :], base[0:HD, :])
            nc.vector.tensor_copy(base3[HD : 2 * HD, 0 : PADL - 132],
                                  base[0:HD, 132:PADL])
            nc.vector.memset(base3[HD : 2 * HD, PADL - 132 : PADL], 0.0)

        mk_pair_copies(padA, padA3)

        # tap-pair matmuls: j=0..2 pairs (t0,t2),(t3,t5),(t6,t8) on base
        # (+2 pairing); j=3 pair (t1,t7) on base3 (+132); j=4 single t4.
        PAIR_OFF = [-67, -1, 65, -66, 0]

        def conv9(dst_tile, src, src3, taps, m_out, tapw, evict):
            total = PW * H
            nch = (total + CS - 1) // CS
            for n in range(nch):
                c0 = PBASE + n * CS
                cw = min(CS, PBASE + total - c0)
                pt = psum.tile([m_out, CS], F32, tag="fe")
                for ti in range(5):
                    off = c0 + PAIR_OFF[ti]
                    if ti < 4:
                        rhs_t = src if ti < 3 else src3
                        MM(pt[:, 0:cw],
                           taps[:, ti * tapw : ti * tapw + m_out],
                           rhs_t[:, off : off + cw],
                           start=(ti == 0), stop=False)
                    else:
                        MM(pt[:, 0:cw],
                           taps[0:HD, ti * tapw : ti * tapw + m_out],
                           src[0:HD, off : off + cw],
                           start=False, stop=True)
                evict(dst_tile[0:m_out, c0 : c0 + cw], pt[:, 0:cw])

        conv9(padB, padA, padA3, d1_taps, HD, HD,
              lambda d, p: nc.scalar.activation(d, p, AF.Copy))

        d1_int = interior(padB)
        ds1 = small.tile([HD, 1], F32, tag="ds1")
        nc.vector.tensor_reduce(ds1[:], d1_int, axis=X.XY, op=OP.add)
        ds2 = small.tile([HD, 1], F32, tag="ds2")
        nc.scalar.activation(interior(padA), d1_int, AF.Square,
                             accum_out=ds2[:])
        packed = small.tile([HD, 2], F32, tag="pk")
        nc.vector.tensor_copy(packed[:, 0:1], ds1[:])
        nc.vector.tensor_copy(packed[:, 1:2], ds2[:])
        nc.sync.dma_start(cc2_in[:], packed[:])
        nc.gpsimd.collective_compute(
            "AllReduce", OP.add, replica_groups=[list(range(8))],
            ins=[cc2_in[:]], outs=[cc2_out[:]])
        red = small.tile([HD, 2], F32, tag="red")
        nc.sync.dma_start(red[:], cc2_out[:])
        sc1, bi1 = bn_scale_bias(red[:, 0:1], red[:, 1:2], 2 * B * L,
                                 d1g_t[:], d1be_t[:], "bn1")

        # h2 into padA interior (pads remain zero), then refresh pair copies
        nc.scalar.activation(interior(padA), d1_int, AF.Relu,
                             bias=bi1[:], scale=sc1[:])
        mk_pair_copies(padA, padA3)
        conv9(out_pad, padA, padA3, d2_taps, NF, NF,
              lambda d, p: nc.scalar.activation(
                  d, p, AF.Identity, bias=d2b_t[:], scale=1.0))
        out_int = out_pad[:NF, PBASE : PBASE + PW * H].rearrange(
            "p (h w) -> p h w", w=PW)[:, :, 0:W]
        nc.sync.dma_start(out_ext[:].rearrange("p (h w) -> p h w", w=W),
                          out_int)

    split_excess_waits(nc)
    return nc


_CACHED = {}


def _get_kernel():
    if "nc" not in _CACHED:
        _CACHED["nc"] = build_kernel()
    return _CACHED["nc"]


def _host_inputs(inputs):
    f32 = np.float32
    bf16 = ml_dtypes.bfloat16
    x = np.asarray(inputs["x"], f32)
    enc_w = np.asarray(inputs["enc_w"], f32)
    in_proj = np.asarray(inputs["in_proj"], f32)
    conv_w = np.asarray(inputs["conv_w"], f32)
    x_proj = np.asarray(inputs["x_proj"], f32)
    dt_w = np.asarray(inputs["dt_w"], f32)
    A_log = np.asarray(inputs["A_log"], f32)
    out_proj = np.asarray(inputs["out_proj"], f32)
    dec1_w = np.asarray(inputs["dec1_w"], f32)
    dec2_w = np.asarray(inputs["dec2_w"], f32)

    xp = np.zeros((B, NB, H + 2, W + 2), f32)
    xp[:, :, 1:-1, 1:-1] = x
    cols = np.empty((NB, 3, 3, B, L), f32)
    for dy in range(3):
        for dx in range(3):
            cols[:, dy, dx] = (
                xp[:, :, dy : dy + H, dx : dx + W]
                .reshape(B, NB, L).transpose(1, 0, 2))
    cols_b = cols.reshape(45, B, L)
    enc_w2 = np.ascontiguousarray(enc_w.reshape(HD, 45).T)

    ip_tap = np.empty((HD, NL, DC, DI), f32)
    ip_z = np.empty((HD, NL, DI), f32)
    wd_T = np.empty((DI, NL, DI), f32)
    bc_full = np.empty((DI, NL, 2 * DS), f32)
    a_full = np.empty((DI, NL, DS), f32)
    op_T = np.empty((DI, NL, HD), f32)
    for i in range(NL):
        for k in range(DC):
            ip_tap[:, i, k, :] = (conv_w[i][:, k : k + 1] * in_proj[i][:DI]).T
        ip_z[:, i, :] = in_proj[i][DI:].T
        wd_T[:, i, :] = (dt_w[i] @ x_proj[i][:DTR]).T
        # B rows negated host-side: kernel stores dlt = -delta, so
        # du = -delta*u and xs = du * (-B) = delta*u*B
        bc_full[:, i, :DS] = -x_proj[i][DTR : DTR + DS].T
        bc_full[:, i, DS:] = x_proj[i][DTR + DS :].T
        a_full[:, i, :] = np.exp(A_log[i])
        op_T[:, i, :] = out_proj[i].T

    # decoder taps in K=128 pairing layout:
    # mm 0..2: rows 0:64 = taps t0/t3/t6, rows 64:128 = taps t2/t5/t8
    # mm 3:    rows 0:64 = t1, rows 64:128 = t7;  mm 4: rows 0:64 = t4
    def pack_taps(wmat, m_out):
        taps = np.zeros((2 * HD, 5, m_out), f32)
        pairs = [(0, 2), (3, 5), (6, 8), (1, 7), (4, None)]
        for j, (ta, tb) in enumerate(pairs):
            dya, dxa = ta // 3, ta % 3
            taps[0:HD, j, :] = wmat[:, :, dya, dxa].T
            if tb is not None:
                dyb, dxb = tb // 3, tb % 3
                taps[HD : 2 * HD, j, :] = wmat[:, :, dyb, dxb].T
        return taps.reshape(2 * HD, 5 * m_out)

    dec1_tap = pack_taps(dec1_w, HD)
    dec2_tap = pack_taps(dec2_w, NF)

    common = {
        "enc_w2": enc_w2.astype(np.float16),
        "enc_g": np.asarray(inputs["enc_g"], f32).reshape(HD, 1),
        "enc_be": np.asarray(inputs["enc_be"], f32).reshape(HD, 1),
        "ip_tap": ip_tap.reshape(HD, NL * DC * DI).astype(np.float16),
        "ip_z": ip_z.reshape(HD, NL * DI).astype(np.float16),
        "conv_b": np.ascontiguousarray(
            np.asarray(inputs["conv_b"], f32).T),           # (DI, NL)
        "wd_T": wd_T.reshape(DI, NL * DI).astype(bf16),
        "dt_b": np.ascontiguousarray(-np.asarray(inputs["dt_b"], f32).T),
        "d_col": np.ascontiguousarray(np.asarray(inputs["Dp"], f32).T) / 2.0,
        "op_T": op_T.reshape(DI, NL * HD).astype(bf16),
        "ident": np.eye(DI, dtype=f32).astype(bf16),
        "dec1_tap": dec1_tap.astype(np.float16),
        "dec1_g": np.asarray(inputs["dec1_g"], f32).reshape(HD, 1),
        "dec1_be": np.asarray(inputs["dec1_be"], f32).reshape(HD, 1),
        "dec2_tap": dec2_tap.astype(np.float16),
        "dec2_b": np.asarray(inputs["dec2_b"], f32).reshape(NF, 1),
    }
    in_maps = []
    for c in range(8):
        b0 = c % B
        sr = (c // B) * DSL
        order = [b0] + [bb for bb in range(B) if bb != b0]
        m = dict(common)
        m["enc_im2col"] = np.ascontiguousarray(
            cols_b[:, order, :].reshape(45, B * L)).astype(np.float16)
        bcs = np.concatenate(
            [bc_full[:, :, sr : sr + DSL],
             bc_full[:, :, DS + sr : DS + sr + DSL]], axis=2)
        m["bc_T"] = np.ascontiguousarray(
            bcs.reshape(DI, NL * 2 * DSL)).astype(bf16)
        m["a_cols"] = np.ascontiguousarray(
            a_full[:, :, sr : sr + DSL].reshape(DI, NL * DSL))
        in_maps.append(m)
    return in_maps


def kernel(**inputs):
    nc = _get_kernel()
    in_maps = _host_inputs(inputs)
    res = run_bass_kernel_spmd(nc, in_maps, core_ids=list(range(8)))
    out = np.empty((B, NF, H, W), np.float32)
    for b_ in range(B):
        out[b_] = res.results[b_]["out"].reshape(NF, H, W)
    return out


if __name__ == "__main__":
    sys.path.insert(0, "/root/problem")
    import reference as ref

    inp = {k: np.asarray(v) for k, v in ref.setup_inputs().items()}
    got = kernel(**inp)
    print("kernel ran, output shape:", got.shape)


# revision 46
# speedup vs baseline: 1.0372x; 1.0372x over previous
"""Trainium2 Bass kernel for MinimalEventMamba (v2).

kernel(**inputs) takes FULL inputs (as from setup_inputs()) and returns the
FULL (4, 10, 64, 64) float32 output. Batch-parallel across 8 NeuronCores
(4 batches x2 replicated, state dim split 8/8 across each pair), one SPMD
Bass launch.

v2 structure (vs v1 baseline at ~1.02ms):
- per-layer work split in sequence halves; front-end (in_proj/dwconv/silu,
  x_proj, softplus via sigmoid+ln) for half h is emitted right before half
  h's scans, so it executes under the previous half's scans.
- scan block is hf-outer / s-inner with per-state carry columns. Per-state
  y contributions (hs*C_s) are accumulated on the TensorEngine via
  identity-matmul PSUM accumulation seeded with the D*u term (no DVE adds);
  gating by silu(z) + out_proj + pair-AllReduce run per quarter and are
  deferred into the next half's scan window (CC flush at s==0, residual
  flush at s==4) so the collective latency hides under scans.
- xs = du*B_s runs on GpSimd; DVE does only scans, hc, yg, du, residual.
- act-table thrash avoided by staging per half: silu -> sigmoid -> ln -> exp.
- encoder convolves only the core's own batch; BN stats via one 8-rank
  AllReduce. decoder convs use K=128 tap-pairing (5 matmuls per 9 taps).
"""
import sys
import types

sys.path.insert(0, "/opt/trn_rl_repo")
sys.path.insert(0, "/opt/trn_rl_repo/concourse")
try:
    from antenv import axon_hooks  # noqa: F401
except ImportError:
    try:
        from trn_agent_boot.trn_boot import _ntff_profile_via_ctypes
        _m = types.ModuleType("antenv.axon_hooks")
        _h = _ntff_profile_via_ctypes("/opt/axon/libaxon_pjrt.so")
        _m.get_axon_ntff_profile_hook = lambda: _h
        _m.set_axon_ntff_profile_hook = lambda h: None
        sys.modules["antenv.axon_hooks"] = _m
    except Exception:
        pass

from contextlib import ExitStack

import numpy as np
import ml_dtypes

import concourse.bass as bass
import concourse.tile as tile
from concourse import mybir
from concourse.bass_utils import run_bass_kernel_spmd
import bass_rust

F32 = mybir.dt.float32
BF16 = mybir.dt.bfloat16
FP16 = mybir.dt.float16

NB, HD, NL, NF = 5, 64, 4, 10
DI, DS, DC, DTR = 128, 16, 4, 4
B, H, W = 4, 64, 64
L = H * W                     # 4096
PW = W + 2                    # padded row stride 66
PADL = PW * (H + 2) + 4       # padded spatial + guard cols (4360)
PBASE = 1 + PW + 1            # first interior col in padded layout
CS = 512
LH = L // 2                   # scan half length
QS = L // 4                   # gate/project/AllReduce quarter
DSL = DS // 2                 # states per core (s-split across core pairs)
XS_ON_GPSIMD = False          # Pool engine can't codegen TensorTensor here


def split_excess_waits(nc, max_waits=1):
    """This container's walrus accepts only 1 sync wait per instruction;
    move overflow waits onto NOPs inserted before the offending op."""
    f = nc.m.functions[0]
    for bb in f.blocks:
        insts = bb.instructions
        i = 0
        while i < len(insts):
            inst = insts[i]
            si = inst.sync_info
            if si is not None and len(si.on_wait) > max_waits:
                waits = list(si.on_wait)
                si.on_wait = waits[-max_waits:]
                inst.sync_info = si
                overflow = waits[:-max_waits]
                eng = nc.engines[inst.engine]
                pos = i
                for j in range(0, len(overflow), max_waits):
                    nop = eng.nop(hint="splitw", nofuse=True)
                    nop_inst = nop.ins if hasattr(nop, "ins") else nop
                    for bb2 in f.blocks:
                        if any(x is nop_inst for x in bb2.instructions):
                            bb2.instructions[:] = [
                                x for x in bb2.instructions if x is not nop_inst
                            ]
                            break
                    nop_inst.sync_info = bass_rust.SyncInfo(
                        on_wait=overflow[j : j + max_waits], on_update=[]
                    )
                    insts.insert(pos, nop_inst)
                    pos += 1
                i = pos + 1
            else:
                i += 1


def build_kernel():
    nc = bass.Bass()
    dp = nc.declare_dram_parameter

    enc_in = dp("enc_im2col", [45, B * L], FP16, isOutput=False)
    enc_w2 = dp("enc_w2", [45, HD], FP16, isOutput=False)
    enc_g = dp("enc_g", [HD, 1], F32, isOutput=False)
    enc_be = dp("enc_be", [HD, 1], F32, isOutput=False)
    ip_tap = dp("ip_tap", [HD, NL * DC * DI], FP16, isOutput=False)
    ip_z = dp("ip_z", [HD, NL * DI], FP16, isOutput=False)
    conv_b = dp("conv_b", [DI, NL], F32, isOutput=False)
    wd_T = dp("wd_T", [DI, NL * DI], BF16, isOutput=False)
    bc_T = dp("bc_T", [DI, NL * 2 * DSL], BF16, isOutput=False)
    dt_b = dp("dt_b", [DI, NL], F32, isOutput=False)
    a_cols = dp("a_cols", [DI, NL * DSL], F32, isOutput=False)
    d_col = dp("d_col", [DI, NL], F32, isOutput=False)
    op_T = dp("op_T", [DI, NL * HD], BF16, isOutput=False)
    ident_p = dp("ident", [DI, DI], BF16, isOutput=False)
    dec1_tap = dp("dec1_tap", [2 * HD, 5 * HD], FP16, isOutput=False)
    dec1_g = dp("dec1_g", [HD, 1], F32, isOutput=False)
    dec1_be = dp("dec1_be", [HD, 1], F32, isOutput=False)
    dec2_tap = dp("dec2_tap", [2 * HD, 5 * NF], FP16, isOutput=False)
    dec2_b = dp("dec2_b", [NF, 1], F32, isOutput=False)

    out_ext = dp("out", [NF, L], F32, isOutput=True)

    bc_dram = nc.dram_tensor("bc_dram", [NL, 2 * DSL, L], BF16)
    y_in = nc.dram_tensor("y_in", [2, HD, LH], BF16)
    y_out = nc.dram_tensor("y_out", [2, HD, LH], BF16)
    cc_in = nc.dram_tensor("cc_in", [HD, 2], F32)
    cc_out = nc.dram_tensor("cc_out", [HD, 2], F32, addr_space="Shared")
    cc2_in = nc.dram_tensor("cc2_in", [HD, 2], F32)
    cc2_out = nc.dram_tensor("cc2_out", [HD, 2], F32, addr_space="Shared")

    ctx = ExitStack()
    with ctx:
        tc = ctx.enter_context(tile.TileContext(nc))
        const = ctx.enter_context(tc.tile_pool(name="const", bufs=1))
        persist = ctx.enter_context(tc.tile_pool(name="persist", bufs=1))
        work = ctx.enter_context(tc.tile_pool(name="work", bufs=1))
        stream = ctx.enter_context(tc.tile_pool(name="stream", bufs=2))
        sloop = ctx.enter_context(tc.tile_pool(name="sloop", bufs=2))
        small = ctx.enter_context(tc.tile_pool(name="small", bufs=1))
        # PSUM: py 4 banks + fe 2x1 + proj 2 = 8 banks exactly
        psum = ctx.enter_context(tc.tile_pool(name="psum", bufs=2, space="PSUM"))
        psumy = ctx.enter_context(tc.tile_pool(name="psumy", bufs=1, space="PSUM"))

        MM = nc.tensor.matmul
        AF = mybir.ActivationFunctionType
        OP = mybir.AluOpType
        X = mybir.AxisListType

        # ---------------- encoder (own batch only) ----------------
        enc_w_t = const.tile([45, HD], FP16)
        nc.sync.dma_start(enc_w_t[:], enc_w2[:])
        enc_g_t = const.tile([HD, 1], F32)
        nc.sync.dma_start(enc_g_t[:], enc_g[:])
        enc_be_t = const.tile([HD, 1], F32)
        nc.sync.dma_start(enc_be_t[:], enc_be[:])

        # all-batch conv on every core: exact BN stats with no collective
        # (own batch's chunks come first in the host-side im2col layout)
        enc_keep = work.tile([HD, L], F32, tag="big")
        s1p = small.tile([HD, 32], F32, tag="s1p")
        s2p = small.tile([HD, 32], F32, tag="s2p")
        for n in range(32):
            cin = stream.tile([45, CS], FP16, tag="enc_cin", bufs=6)
            nc.sync.dma_start(cin[:], enc_in[:, bass.ts(n, CS)])
            pt = psum.tile([HD, CS], F32,
                           tag=("fe" if n % 3 else "proj"),
                           bufs=(2 if n % 3 else 1))
            MM(pt[:], enc_w_t[:], cin[:], start=True, stop=True)
            if n < 8:
                dst = enc_keep[:, bass.ts(n, CS)]
            else:
                scr = stream.tile([HD, CS], F32, tag="enc_scr")
                dst = scr[:]
            nc.scalar.activation(dst, pt[:], AF.Copy,
                                 accum_out=s1p[:, n : n + 1])
            sq = stream.tile([HD, CS], F32, tag="enc_sq")
            nc.vector.scalar_tensor_tensor(sq[:], dst, 1.0, dst,
                                           OP.mult, OP.mult,
                                           accum_out=s2p[:, n : n + 1])
        red0 = small.tile([HD, 2], F32, tag="red0")
        nc.vector.tensor_reduce(red0[:, 0:1], s1p[:], axis=X.X, op=OP.add)
        nc.vector.tensor_reduce(red0[:, 1:2], s2p[:], axis=X.X, op=OP.add)

        def bn_scale_bias(s1ap, s2ap, n_elems, g_ap, be_ap, tag):
            inv_n = 1.0 / n_elems
            mean = small.tile([HD, 1], F32, tag=tag + "m")
            nc.vector.tensor_scalar_mul(mean[:], s1ap, inv_n)
            m2 = small.tile([HD, 1], F32, tag=tag + "m2")
            nc.vector.tensor_tensor(m2[:], mean[:], mean[:], OP.mult)
            var = small.tile([HD, 1], F32, tag=tag + "v")
            nc.vector.scalar_tensor_tensor(var[:], s2ap, inv_n, m2[:],
                                           OP.mult, OP.subtract)
            veps = small.tile([HD, 1], F32, tag=tag + "ve")
            nc.vector.tensor_scalar_add(veps[:], var[:], 1e-5)
            rv = small.tile([HD, 1], F32, tag=tag + "rv")
            nc.vector.reciprocal(rv[:], veps[:])
            rstd = small.tile([HD, 1], F32, tag=tag + "rs")
            nc.scalar.activation(rstd[:], rv[:], AF.Sqrt)
            scale = small.tile([HD, 1], F32, tag=tag + "sc")
            nc.vector.tensor_tensor(scale[:], g_ap, rstd[:], OP.mult)
            nscale = small.tile([HD, 1], F32, tag=tag + "ns")
            nc.vector.tensor_scalar_mul(nscale[:], scale[:], -1.0)
            bias = small.tile([HD, 1], F32, tag=tag + "bi")
            nc.vector.scalar_tensor_tensor(bias[:], mean[:], nscale[:], be_ap,
                                           OP.mult, OP.add)
            return scale, bias

        sc0, bi0 = bn_scale_bias(red0[:, 0:1], red0[:, 1:2], B * L,
                                 enc_g_t[:], enc_be_t[:], "bn0")

        t_t = persist.tile([HD, 4 + L], F32)
        nc.vector.memset(t_t[:, 0:4], 0.0)
        t16 = persist.tile([HD, 4 + L], FP16)
        nc.vector.memset(t16[:, 0:4], 0.0)
        for hh in range(2):
            hsl = slice(4 + hh * LH, 4 + (hh + 1) * LH)
            nc.scalar.activation(t_t[:, hsl], enc_keep[:, hh * LH : (hh + 1) * LH],
                                 AF.Relu, bias=bi0[:], scale=sc0[:])
            nc.scalar.activation(t16[:, hsl], t_t[:, hsl], AF.Copy)

        # ---------------- trunk weights ----------------
        iptap_t = const.tile([HD, NL * DC * DI], FP16)
        nc.sync.dma_start(iptap_t[:], ip_tap[:])
        ipz_t = const.tile([HD, NL * DI], FP16)
        nc.sync.dma_start(ipz_t[:], ip_z[:])
        convb_t = const.tile([DI, NL], F32)
        nc.sync.dma_start(convb_t[:], conv_b[:])
        wd_t = const.tile([DI, NL * DI], BF16)
        nc.sync.dma_start(wd_t[:], wd_T[:])
        bct_t = const.tile([DI, NL * 2 * DSL], BF16)
        nc.sync.dma_start(bct_t[:], bc_T[:])
        dtb_t = const.tile([DI, NL], F32)
        nc.sync.dma_start(dtb_t[:], dt_b[:])
        acols_t = const.tile([DI, NL * DSL], F32)
        nc.sync.dma_start(acols_t[:], a_cols[:])
        dcol_t = const.tile([DI, NL], F32)
        nc.sync.dma_start(dcol_t[:], d_col[:])
        opt_t = const.tile([DI, NL * HD], BF16)
        nc.sync.dma_start(opt_t[:], op_T[:])
        ident_t = const.tile([DI, DI], BF16)
        nc.sync.dma_start(ident_t[:], ident_p[:])

        # ---------------- trunk ----------------
        # deferred per-quarter tails: projection+evict+CC flush at the next
        # half's s==0 (PE/Act/Pool streams run them under the scans);
        # dtsum+residual+t16 at the next half's s==5 (AR has landed by then)
        pend_cc = []
        pend_post = []

        def flush(lst):
            for fn in lst:
                fn()
            lst.clear()

        for li in range(NL):
            xi_c = work.tile([DI, L], BF16, tag="xi_c")
            sz = work.tile([DI, L], BF16, tag="sz")
            dlt = work.tile([DI, L], BF16, tag="dlt")
            du = work.tile([DI, L], BF16, tag="du")
            bc_sb = work.tile([2 * DSL, L], BF16, tag="bc_sb")
            dt_part = work.tile([HD, L], BF16, tag="dt_part")
            dtsum = work.tile([HD, L], BF16, tag="dtsum")
            carry = small.tile([DI, DSL], BF16, tag="carry")

            for hf in range(2):
                sl = slice(hf * LH, (hf + 1) * LH)
                # ---- front-end for this half (chunks 4hf .. 4hf+3) ----
                # stage A: silu table
                for n in range(4 * hf, 4 * hf + 4):
                    p_xi = psum.tile([DI, CS], F32, tag="fe")
                    for k in range(DC):
                        MM(p_xi[:],
                           iptap_t[:, (li * DC + k) * DI : (li * DC + k + 1) * DI],
                           t16[:, 1 + k + n * CS : 1 + k + n * CS + CS],
                           start=(k == 0), stop=(k == DC - 1))
                    nc.scalar.activation(xi_c[:, bass.ts(n, CS)], p_xi[:],
                                         AF.Silu,
                                         bias=convb_t[:, li : li + 1], scale=1.0)
                    p_z = psum.tile([DI, CS], F32, tag="fe")
                    MM(p_z[:], ipz_t[:, li * DI : (li + 1) * DI],
                       t16[:, 4 + n * CS : 4 + (n + 1) * CS],
                       start=True, stop=True)
                    nc.scalar.activation(sz[:, bass.ts(n, CS)], p_z[:], AF.Silu)
                # stage B: sigmoid table (+ bc Copy evictions)
                sg = work.tile([DI, LH], F32, tag="big")
                for j in range(4):
                    n = 4 * hf + j
                    p_d = psum.tile([DI, CS], F32, tag="fe")
                    MM(p_d[:], wd_t[:, li * DI : (li + 1) * DI],
                       xi_c[:, bass.ts(n, CS)], start=True, stop=True)
                    nc.scalar.activation(sg[:, bass.ts(j, CS)], p_d[:],
                                         AF.Sigmoid,
                                         bias=dtb_t[:, li : li + 1], scale=-1.0)
                    p_bc = psum.tile([2 * DSL, CS], F32, tag="fe")
                    MM(p_bc[:], bct_t[:, li * 2 * DSL : (li + 1) * 2 * DSL],
                       xi_c[:, bass.ts(n, CS)], start=True, stop=True)
                    nc.scalar.activation(bc_sb[:, bass.ts(n, CS)], p_bc[:],
                                         AF.Copy)
                    nc.sync.dma_start(bc_dram[li][:, bass.ts(n, CS)],
                                      bc_sb[:, bass.ts(n, CS)])
                # stage C: ln table; du on DVE
                for j in range(4):
                    n = 4 * hf + j
                    nc.scalar.activation(dlt[:, bass.ts(n, CS)],
                                         sg[:, bass.ts(j, CS)], AF.Ln)
                    nc.vector.tensor_tensor(du[:, bass.ts(n, CS)],
                                            dlt[:, bass.ts(n, CS)],
                                            xi_c[:, bass.ts(n, CS)], OP.mult)

                # ---- scan block ----
                # previous half's projection + exchange first: its Act/PE ops
                # land ahead of dterm/exp so the AllReduce input chain starts
                # as early as possible (AR end-to-end latency is ~40us)
                flush(pend_cc)
                # id-matmul accumulation covers dterm + states 0..5; the last
                # two states fold into yg on DVE so late idMMs don't block
                # the next half's FE matmuls on the in-order PE queue.
                dterm = sloop.tile([DI, LH], BF16, tag="dterm", bufs=1)
                nc.scalar.activation(dterm[:], xi_c[:, sl], AF.Copy,
                                     scale=dcol_t[:, li : li + 1])
                p_y = psumy.tile([DI, LH], F32, tag="py")
                for c in range(LH // CS):
                    MM(p_y[:, bass.ts(c, CS)], ident_t[:],
                       dterm[:, bass.ts(c, CS)], start=True, stop=False)
                hc_last = [None, None]
                for s in range(DSL):
                    dA = sloop.tile([DI, LH], FP16, tag="dA", bufs=4)
                    nc.scalar.activation(
                        dA[:], dlt[:, sl], AF.Exp,
                        scale=acols_t[:, li * DSL + s : li * DSL + s + 1])
                    brep = sloop.tile([DI, LH], BF16, tag="brep", bufs=3)
                    for ph in range(2):
                        nc.sync.dma_start(
                            brep[64 * ph : 64 * ph + 64, :],
                            bc_dram[li][s : s + 1, sl].broadcast_to((64, LH)))
                    crep = sloop.tile([DI, LH], BF16, tag="crep", bufs=2)
                    for ph in range(2):
                        nc.sync.dma_start(
                            crep[64 * ph : 64 * ph + 64, :],
                            bc_dram[li][DSL + s : DSL + s + 1, sl].broadcast_to(
                                (64, LH)))
                    xs = sloop.tile([DI, LH], BF16, tag="xs")
                    nc.vector.tensor_tensor(xs[:], du[:, sl], brep[:], OP.mult)
                    hs = sloop.tile([DI, LH], BF16, tag="hs")
                    init = 0.0 if hf == 0 else carry[:, s : s + 1]
                    nc.vector.tensor_tensor_scan(hs[:], dA[:], xs[:], init,
                                                 OP.mult, OP.add)
                    if hf == 0:
                        nc.vector.tensor_copy(carry[:, s : s + 1],
                                              hs[:, LH - 1 : LH])
                    if s < DSL - 2:
                        hc = sloop.tile([DI, LH], BF16, tag="hc")
                        nc.vector.tensor_tensor(hc[:], hs[:], crep[:], OP.mult)
                        for c in range(LH // CS):
                            MM(p_y[:, bass.ts(c, CS)], ident_t[:],
                               hc[:, bass.ts(c, CS)], start=False,
                               stop=(s == DSL - 3))
                    else:
                        hc = sloop.tile([DI, LH], BF16, tag="hcl", bufs=2)
                        nc.vector.tensor_tensor(hc[:], hs[:], crep[:], OP.mult)
                        hc_last[s - (DSL - 2)] = hc
                    if s == 5:
                        flush(pend_post)
                # ---- gate (+ last two states) per quarter, then one
                # projection + AllReduce per half (fewer collectives: the CC
                # pipeline is latency-bound, ~20us per collective) ----
                ygs = []
                for qq in range(2):
                    q = 2 * hf + qq
                    qs = slice(q * QS, (q + 1) * QS)
                    cq = slice(qq * QS, (qq + 1) * QS)
                    ysum = sloop.tile([DI, QS], BF16, tag="ysum")
                    nc.vector.scalar_tensor_tensor(ysum[:], p_y[:, cq], 1.0,
                                                   hc_last[0][:, cq],
                                                   OP.mult, OP.add)
                    nc.vector.tensor_tensor(ysum[:], ysum[:],
                                            hc_last[1][:, cq], OP.add)
                    yg = sloop.tile([DI, QS], BF16, tag="yg")
                    nc.vector.tensor_tensor(yg[:], ysum[:], sz[:, qs], OP.mult)
                    ygs.append(yg)

                def mk_cc(li=li, hf=hf, sl=sl, ygs=ygs, dtsum=dtsum,
                          dt_part=dt_part):
                    def go():
                        for qq in range(2):
                            q = 2 * hf + qq
                            qs = slice(q * QS, (q + 1) * QS)
                            p_o = psum.tile([HD, QS], F32, tag="proj", bufs=1,
                                            name=f"p_o_{li}_{q}")
                            for c in range(QS // CS):
                                MM(p_o[:, bass.ts(c, CS)],
                                   opt_t[:, li * HD : (li + 1) * HD],
                                   ygs[qq][:, bass.ts(c, CS)],
                                   start=True, stop=True)
                            nc.scalar.activation(dt_part[:, qs], p_o[:],
                                                 AF.Copy)
                        for rb in range(4):
                            nc.sync.dma_start(
                                y_in[hf][16 * rb : 16 * rb + 16, :],
                                dt_part[16 * rb : 16 * rb + 16, sl])
                        nc.gpsimd.collective_compute(
                            "AllReduce", OP.add,
                            replica_groups=[[0, 4], [1, 5], [2, 6], [3, 7]],
                            ins=[y_in[hf]], outs=[y_out[hf]])
                        for rb in range(4):
                            nc.sync.dma_start(
                                dtsum[16 * rb : 16 * rb + 16, sl],
                                y_out[hf][16 * rb : 16 * rb + 16, :])
                    return go

                def mk_post(li=li, hf=hf, sl=sl, dtsum=dtsum):
                    def go():
                        tsl = slice(4 + hf * LH, 4 + (hf + 1) * LH)
                        nc.vector.tensor_tensor(t_t[:, tsl], t_t[:, tsl],
                                                dtsum[:, sl], OP.add)
                        if li < NL - 1:
                            nc.scalar.activation(t16[:, tsl], t_t[:, tsl],
                                                 AF.Copy)
                    return go

                pend_cc.append(mk_cc())
                pend_post.append(mk_post())

        flush(pend_cc)
        flush(pend_post)

        # ---------------- decoder ----------------
        d1_taps = const.tile([2 * HD, 5 * HD], FP16)
        nc.sync.dma_start(d1_taps[:], dec1_tap[:])
        d2_taps = const.tile([2 * HD, 5 * NF], FP16)
        nc.sync.dma_start(d2_taps[:], dec2_tap[:])
        d1g_t = const.tile([HD, 1], F32)
        nc.sync.dma_start(d1g_t[:], dec1_g[:])
        d1be_t = const.tile([HD, 1], F32)
        nc.sync.dma_start(d1be_t[:], dec1_be[:])
        d2b_t = const.tile([NF, 1], F32)
        nc.sync.dma_start(d2b_t[:], dec2_b[:])

        # padA/padB: rows 0:HD = base, rows HD:2HD = base shifted +2 cols.
        # padA3/padB3: rows 0:HD = base copy, rows HD:2HD = base shifted +132.
        padA = work.tile([2 * HD, PADL], FP16, tag="xi_c")
        nc.vector.memset(padA[0:HD, :], 0.0)
        padA3 = work.tile([2 * HD, PADL], FP16, tag="sz")
        padB = work.tile([HD, PADL], FP16, tag="dt_part")
        out_pad = work.tile([NF, PADL], F32, tag="big")

        def interior(tile_ap):
            return tile_ap[0:HD, PBASE : PBASE + PW * H].rearrange(
                "p (h w) -> p h w", w=PW)[:, :, 0:W]

        nc.scalar.activation(interior(padA),
                             t_t[:, 4:].rearrange("p (h w) -> p h w", w=W),
                             AF.Copy)

        def mk_pair_copies(base, base3):
            nc.vector.tensor_copy(base[HD : 2 * HD, 0 : PADL - 2],
                                  base[0:HD, 2:PADL])
            nc.vector.memset(base[HD : 2 * HD, PADL - 2 : PADL], 0.0)
            nc.vector.tensor_copy(base3[0:HD, :], base[0:HD, :])
            nc.vector.tensor_copy(base3[HD : 2 * HD, 0 : PADL - 132],
                                  base[0:HD, 132:PADL])
            nc.vector.memset(base3[HD : 2 * HD, PADL - 132 : PADL], 0.0)

        mk_pair_copies(padA, padA3)

        # tap-pair matmuls: j=0..2 pairs (t0,t2),(t3,t5),(t6,t8) on base
        # (+2 pairing); j=3 pair (t1,t7) on base3 (+132); j=4 single t4.
        PAIR_OFF = [-67, -1, 65, -66, 0]

        def conv9(dst_tile, src, src3, taps, m_out, tapw, evict):
            total = PW * H
            nch = (total + CS - 1) // CS
            for n in range(nch):
                c0 = PBASE + n * CS
                cw = min(CS, PBASE + total - c0)
                pt = psum.tile([m_out, CS], F32, tag="fe")
                for ti in range(5):
                    off = c0 + PAIR_OFF[ti]
                    if ti < 4:
                        rhs_t = src if ti < 3 else src3
                        MM(pt[:, 0:cw],
                           taps[:, ti * tapw : ti * tapw + m_out],
                           rhs_t[:, off : off + cw],
                           start=(ti == 0), stop=False)
                    else:
                        MM(pt[:, 0:cw],
                           taps[0:HD, ti * tapw : ti * tapw + m_out],
                           src[0:HD, off : off + cw],
                           start=False, stop=True)
                evict(dst_tile[0:m_out, c0 : c0 + cw], pt[:, 0:cw])

        conv9(padB, padA, padA3, d1_taps, HD, HD,
              lambda d, p: nc.scalar.activation(d, p, AF.Copy))

        d1_int = interior(padB)
        ds1 = small.tile([HD, 1], F32, tag="ds1")
        nc.vector.tensor_reduce(ds1[:], d1_int, axis=X.XY, op=OP.add)
        ds2 = small.tile([HD, 1], F32, tag="ds2")
        nc.scalar.activation(interior(padA), d1_int, AF.Square,
                             accum_out=ds2[:])
        packed = small.tile([HD, 2], F32, tag="pk")
        nc.vector.tensor_copy(packed[:, 0:1], ds1[:])
        nc.vector.tensor_copy(packed[:, 1:2], ds2[:])
        nc.sync.dma_start(cc2_in[:], packed[:])
        nc.gpsimd.collective_compute(
            "AllReduce", OP.add, replica_groups=[list(range(8))],
            ins=[cc2_in[:]], outs=[cc2_out[:]])
        red = small.tile([HD, 2], F32, tag="red")
        nc.sync.dma_start(red[:], cc2_out[:])
        sc1, bi1 = bn_scale_bias(red[:, 0:1], red[:, 1:2], 2 * B * L,
                                 d1g_t[:], d1be_t[:], "bn1")

        # h2 into padA interior (pads remain zero), then refresh pair copies
        nc.scalar.activation(interior(padA), d1_int, AF.Relu,
                             bias=bi1[:], scale=sc1[:])
        mk_pair_copies(padA, padA3)
        conv9(out_pad, padA, padA3, d2_taps, NF, NF,
              lambda d, p: nc.scalar.activation(
                  d, p, AF.Identity, bias=d2b_t[:], scale=1.0))
        out_int = out_pad[:NF, PBASE : PBASE + PW * H].rearrange(
            "p (h w) -> p h w", w=PW)[:, :, 0:W]
        nc.sync.dma_start(out_ext[:].rearrange("p (h w) -> p h w", w=W),
                          out_int)

    split_excess_waits(nc)
    return nc


_CACHED = {}


def _get_kernel():
    if "nc" not in _CACHED:
        _CACHED["nc"] = build_kernel()
    return _CACHED["nc"]


def _host_inputs(inputs):
    f32 = np.float32
    bf16 = ml_dtypes.bfloat16
    x = np.asarray(inputs["x"], f32)
    enc_w = np.asarray(inputs["enc_w"], f32)
    in_proj = np.asarray(inputs["in_proj"], f32)
    conv_w = np.asarray(inputs["conv_w"], f32)
    x_proj = np.asarray(inputs["x_proj"], f32)
    dt_w = np.asarray(inputs["dt_w"], f32)
    A_log = np.asarray(inputs["A_log"], f32)
    out_proj = np.asarray(inputs["out_proj"], f32)
    dec1_w = np.asarray(inputs["dec1_w"], f32)
    dec2_w = np.asarray(inputs["dec2_w"], f32)

    xp = np.zeros((B, NB, H + 2, W + 2), f32)
    xp[:, :, 1:-1, 1:-1] = x
    cols = np.empty((NB, 3, 3, B, L), f32)
    for dy in range(3):
        for dx in range(3):
            cols[:, dy, dx] = (
                xp[:, :, dy : dy + H, dx : dx + W]
                .reshape(B, NB, L).transpose(1, 0, 2))
    cols_b = cols.reshape(45, B, L)
    enc_w2 = np.ascontiguousarray(enc_w.reshape(HD, 45).T)

    ip_tap = np.empty((HD, NL, DC, DI), f32)
    ip_z = np.empty((HD, NL, DI), f32)
    wd_T = np.empty((DI, NL, DI), f32)
    bc_full = np.empty((DI, NL, 2 * DS), f32)
    a_full = np.empty((DI, NL, DS), f32)
    op_T = np.empty((DI, NL, HD), f32)
    for i in range(NL):
        for k in range(DC):
            ip_tap[:, i, k, :] = (conv_w[i][:, k : k + 1] * in_proj[i][:DI]).T
        ip_z[:, i, :] = in_proj[i][DI:].T
        wd_T[:, i, :] = (dt_w[i] @ x_proj[i][:DTR]).T
        # B rows negated host-side: kernel stores dlt = -delta, so
        # du = -delta*u and xs = du * (-B) = delta*u*B
        bc_full[:, i, :DS] = -x_proj[i][DTR : DTR + DS].T
        bc_full[:, i, DS:] = x_proj[i][DTR + DS :].T
        a_full[:, i, :] = np.exp(A_log[i])
        op_T[:, i, :] = out_proj[i].T

    # decoder taps in K=128 pairing layout:
    # mm 0..2: rows 0:64 = taps t0/t3/t6, rows 64:128 = taps t2/t5/t8
    # mm 3:    rows 0:64 = t1, rows 64:128 = t7;  mm 4: rows 0:64 = t4
    def pack_taps(wmat, m_out):
        taps = np.zeros((2 * HD, 5, m_out), f32)
        pairs = [(0, 2), (3, 5), (6, 8), (1, 7), (4, None)]
        for j, (ta, tb) in enumerate(pairs):
            dya, dxa = ta // 3, ta % 3
            taps[0:HD, j, :] = wmat[:, :, dya, dxa].T
            if tb is not None:
                dyb, dxb = tb // 3, tb % 3
                taps[HD : 2 * HD, j, :] = wmat[:, :, dyb, dxb].T
        return taps.reshape(2 * HD, 5 * m_out)

    dec1_tap = pack_taps(dec1_w, HD)
    dec2_tap = pack_taps(dec2_w, NF)

    common = {
        "enc_w2": enc_w2.astype(np.float16),
        "enc_g": np.asarray(inputs["enc_g"], f32).reshape(HD, 1),
        "enc_be": np.asarray(inputs["enc_be"], f32).reshape(HD, 1),
        "ip_tap": ip_tap.reshape(HD, NL * DC * DI).astype(np.float16),
        "ip_z": ip_z.reshape(HD, NL * DI).astype(np.float16),
        "conv_b": np.ascontiguousarray(
            np.asarray(inputs["conv_b"], f32).T),           # (DI, NL)
        "wd_T": wd_T.reshape(DI, NL * DI).astype(bf16),
        "dt_b": np.ascontiguousarray(-np.asarray(inputs["dt_b"], f32).T),
        "d_col": np.ascontiguousarray(np.asarray(inputs["Dp"], f32).T) / 2.0,
        "op_T": op_T.reshape(DI, NL * HD).astype(bf16),
        "ident": np.eye(DI, dtype=f32).astype(bf16),
        "dec1_tap": dec1_tap.astype(np.float16),
        "dec1_g": np.asarray(inputs["dec1_g"], f32).reshape(HD, 1),
        "dec1_be": np.asarray(inputs["dec1_be"], f32).reshape(HD, 1),
        "dec2_tap": dec2_tap.astype(np.float16),
        "dec2_b": np.asarray(inputs["dec2_b"], f32).reshape(NF, 1),
    }
    in_maps = []
    for c in range(8):
        b0 = c % B
        sr = (c // B) * DSL
        order = [b0] + [bb for bb in range(B) if bb != b0]
        m = dict(common)
        m["enc_im2col"] = np.ascontiguousarray(
            cols_b[:, order, :].reshape(45, B * L)).astype(np.float16)
        bcs = np.concatenate(
            [bc_full[:, :, sr : sr + DSL],
             bc_full[:, :, DS + sr : DS + sr + DSL]], axis=2)
        m["bc_T"] = np.ascontiguousarray(
            bcs.reshape(DI, NL * 2 * DSL)).astype(bf16)
        m["a_cols"] = np.ascontiguousarray(
            a_full[:, :, sr : sr + DSL].reshape(DI, NL * DSL))
        in_maps.append(m)
    return in_maps


def kernel(**inputs):
    nc = _get_kernel()
    in_maps = _host_inputs(inputs)
    res = run_bass_kernel_spmd(nc, in_maps, core_ids=list(range(8)))
    out = np.empty((B, NF, H, W), np.float32)
    for b_ in range(B):
        out[b_] = res.results[b_]["out"].reshape(NF, H, W)
    return out


if __name__ == "__main__":
    sys.path.insert(0, "/root/problem")
    import reference as ref

    inp = {k: np.asarray(v) for k, v in ref.setup_inputs().items()}
    got = kernel(**inp)
    print("kernel ran, output shape:", got.shape)
